# revision 2
# baseline (speedup 1.0000x reference)
"""Trainium2 Bass kernel for nn_Decoder — batch-sharded, zero per-step collectives.

Sharding (8 cores): each core owns NB=8 batch rows end-to-end. All weights
replicated. The T=32 recurrence runs fully local per core; the only
collective is one AllReduce for the h0/c0 init (contraction sharded 8x).

Layouts put the wide dims (hidden/gates/attn) on partitions and the 8 local
batch rows on the free axis, so per-step matmuls stream only 2-8 columns
(the big operand is stationary). Final fc runs post-loop as [tn, vocab]
with h_all as the stationary operand.
"""

import sys

sys.path.insert(0, "/opt/trn_rl_repo")

import numpy as np
import ml_dtypes

import concourse.bass as bass
import concourse.bacc as bacc
import concourse.mybir as mybir
import concourse.tile as tile
from concourse.bass_utils import run_bass_kernel_spmd

F32 = mybir.dt.float32
BF16 = mybir.dt.bfloat16
AX = mybir.AxisListType
OP = mybir.AluOpType
ACT = mybir.ActivationFunctionType

N, P, ENC = 64, 64, 1536
T = 32
V, E, A, H = 10000, 512, 512, 512
NC_ = 8
NB = N // NC_        # 8 local batch rows
CT = ENC // 128      # 12 contraction tiles over ENC
KS = P * ENC // NC_  # 12288 init contraction rows per core
G4 = 4 * H           # 2048 gates
NPL = NB * P         # 512 local (n,p) positions
NM = NB // 2         # 4 n-pairs

bf16 = ml_dtypes.bfloat16

# sim-probe flags (always True in production)
DO_LOOP = True
DO_FC = True
DO_INIT = True
DO_PRE = True
NSTEP = T

_cache = {}


def _build():
    nc = bacc.Bacc(None, target_bir_lowering=False, debug=False)

    # ---- I/O (all per-core shards prepared on host) ----
    featT_loc = nc.dram_tensor("featT_loc", [128, CT, NPL], BF16, kind="ExternalInput")
    w_ihcT = nc.dram_tensor("w_ihcT", [128, CT, G4], BF16, kind="ExternalInput")
    w_ieT = nc.dram_tensor("w_ieT", [128, 4, G4], BF16, kind="ExternalInput")
    w_hhT = nc.dram_tensor("w_hhT", [128, 4, G4], BF16, kind="ExternalInput")
    b_ehT = nc.dram_tensor("b_ehT", [128, 16], F32, kind="ExternalInput")
    embT_loc = nc.dram_tensor("embT_loc", [128, 4, T * NB], BF16, kind="ExternalInput")
    tok_WT = nc.dram_tensor("tok_WT", [128, 4, A], BF16, kind="ExternalInput")
    feat_WT = nc.dram_tensor("feat_WT", [128, CT, A], BF16, kind="ExternalInput")
    b_attn = nc.dram_tensor("b_attn", [128, 4], F32, kind="ExternalInput")
    w_full = nc.dram_tensor("w_full", [128, 4], BF16, kind="ExternalInput")
    masks = nc.dram_tensor("masks", [128, 2], BF16, kind="ExternalInput")
    feat_initT = nc.dram_tensor("feat_initT", [128, 96, N], BF16, kind="ExternalInput")
    initW = nc.dram_tensor("initW", [128, 96, 1024], BF16, kind="ExternalInput")
    fc_WTd = nc.dram_tensor("fc_WT", [128, 4, V], BF16, kind="ExternalInput")
    fc_bd = nc.dram_tensor("fc_b", [1, V], BF16, kind="ExternalInput")
    ones1d = nc.dram_tensor("ones1", [1, 128], F32, kind="ExternalInput")
    identd = nc.dram_tensor("ident", [128, 128], F32, kind="ExternalInput")
    preds_out = nc.dram_tensor("preds", [T * NB, V], F32, kind="ExternalOutput")

    rank = nc.partition_id()

    with tile.TileContext(nc) as tc:
        with (
            tc.tile_pool(name="pp", bufs=1) as pp,
            tc.tile_pool(name="dram", bufs=1, space="DRAM") as dram,
        ):
            # small persistent tensors — tiny DMAs issued first
            tokW = pp.tile([128, 4, A], BF16)
            nc.sync.dma_start(tokW[:], tok_WT[:])
            battn = pp.tile([128, 4], F32)
            nc.sync.dma_start(battn[:], b_attn[:])
            wfull = pp.tile([128, 4], BF16)
            nc.sync.dma_start(wfull[:], w_full[:])
            msk = pp.tile([128, 2], BF16)
            nc.sync.dma_start(msk[:], masks[:])
            behT = pp.tile([128, 16], F32)
            nc.sync.dma_start(behT[:], b_ehT[:])
            ones1 = pp.tile([1, 128], F32)
            nc.sync.dma_start(ones1[:], ones1d[:])
            idn = pp.tile([128, 128], F32)
            nc.sync.dma_start(idn[:], identd[:])

            featW = pp.tile([128, CT, A], BF16)
            whh = pp.tile([128, 4, G4], BF16)
            attn_img = pp.tile([128, 4, NPL], BF16)   # [a_lo, a_hi, (n,p)]
            G_sb = pp.tile([128, NM, G4], BF16)       # [(j,p), m, g]
            emb_g = pp.tile([128, 16, T * NB], BF16)  # [g_lo, g_hi, (t,n)] (+b_eh)
            h_all = pp.tile([128, 4, T + 1, NB], BF16)  # [h_lo, h_hi, t, n]
            c_a = pp.tile([128, 4, NB], F32)
            c_b = pp.tile([128, 4, NB], F32)
            alpha_pack = pp.tile([128, NM, 2], BF16)  # masked normalized alphas
            nc.vector.memset(alpha_pack[:], 0.0)
            hc_loc = pp.tile([128, 8, NB], F32)       # h0 local cols
            c_st = pp.tile([64, 8, NB], F32)          # c0 transposed staging

            with (
                tc.tile_pool(name="pre", bufs=1) as pf,
                tc.tile_pool(name="psp", bufs=1, space="PSUM") as psp,
            ):
                # -------- init h0/c0 DMAs first: the AllReduce is the long pole
                finit = pf.tile([128, 96, N], BF16)
                nc.sync.dma_start(finit[:], feat_initT[:])
                NCH = 8
                with tc.tile_pool(name="initw", bufs=2) as piw:
                    with tc.tile_pool(name="psi", bufs=1, space="PSUM") as psi:
                        ps_h0 = psi.tile([64, 512], F32)
                        ps_c0 = psi.tile([64, 512], F32)
                        for ch in range(96 // NCH):
                            iw = piw.tile([128, NCH, 1024], BF16, tag="iw")
                            nc.sync.dma_start(
                                iw[:], initW[:, ch * NCH : (ch + 1) * NCH, :]
                            )
                            if DO_INIT:
                                for k in range(NCH):
                                    kt = ch * NCH + k
                                    nc.tensor.matmul(
                                        ps_h0[:], finit[:, kt, :],
                                        iw[:, k, 0:512],
                                        start=(kt == 0), stop=(kt == 95),
                                    )
                                    nc.tensor.matmul(
                                        ps_c0[:], finit[:, kt, :],
                                        iw[:, k, 512:1024],
                                        start=(kt == 0), stop=(kt == 95),
                                    )
                        if DO_INIT:
                            h_ev = pf.tile([64, 512], F32)
                            nc.vector.tensor_copy(h_ev[:], ps_h0[:])
                            c_ev = pf.tile([64, 512], F32)
                            nc.vector.tensor_copy(c_ev[:], ps_c0[:])
                            hc_sb = pf.tile([128, 12, 64], F32)
                            with tc.tile_pool(
                                name="pst", bufs=1, space="PSUM"
                            ) as pst:
                                ps_ht = pst.tile([128, 4, 64], F32)
                                for q in range(4):
                                    nc.tensor.transpose(
                                        ps_ht[:, q, :],
                                        h_ev[:, 128 * q : 128 * (q + 1)],
                                        idn[0:64, 0:64],
                                    )
                                nc.vector.tensor_copy(hc_sb[:, 0:4, :], ps_ht[:])
                                ps_ct = pst.tile([64, 8, 64], F32)
                                for k8 in range(8):
                                    nc.tensor.transpose(
                                        ps_ct[:, k8, :],
                                        c_ev[:, 64 * k8 : 64 * (k8 + 1)],
                                        idn[0:64, 0:64],
                                    )
                                nc.vector.memset(hc_sb[64:128, 4:12, :], 0.0)
                                nc.vector.tensor_copy(
                                    hc_sb[0:64, 4:12, :], ps_ct[:]
                                )
                            hc_in = dram.tile([128, 12, 64], F32)
                            hc_out = dram.tile([128, 12, 64], F32)
                            nc.sync.dma_start(hc_in[:], hc_sb[:])
                            nc.gpsimd.collective_compute(
                                "AllReduce", OP.add,
                                replica_groups=[list(range(NC_))],
                                ins=[hc_in[:]], outs=[hc_out[:]],
                            )
                            # h0 [h_lo, h_hi, n_loc]
                            for i in range(4):
                                nc.sync.dma_start(
                                    hc_loc[:, i, :],
                                    hc_out[:, i, bass.ts(rank, NB)],
                                )
                            # c0 [c_lo(64), k8, n_loc] -> c_a [128, 4, 8]
                            for k8 in range(8):
                                nc.sync.dma_start(
                                    c_st[:, k8, :],
                                    hc_out[0:64, 4 + k8, bass.ts(rank, NB)],
                                )

                    # remaining pre-loop DMAs, in consumption order
                    ftl = pf.tile([128, CT, NPL], BF16)
                    nc.sync.dma_start(ftl[:], featT_loc[:])
                    nc.sync.dma_start(featW[:], feat_WT[:])
                    embl = pf.tile([128, 4, T * NB], BF16)
                    nc.sync.dma_start(embl[:], embT_loc[:])
                    wie = pf.tile([128, 4, G4], BF16)
                    nc.sync.dma_start(wie[:], w_ieT[:])
                    wihc = pf.tile([128, CT, G4], BF16)
                    for gc in range(4):
                        nc.sync.dma_start(
                            wihc[:, :, 512 * gc : 512 * (gc + 1)],
                            w_ihcT[:, :, 512 * gc : 512 * (gc + 1)],
                        )
                    nc.sync.dma_start(whh[:], w_hhT[:])

                    # -------- precompute mms (PE, run while initW DMA streams)
                    # attn_img[a, np] = feat_W.T @ featT_loc (+ attn bias)
                    for q in range(4 if DO_PRE else 0):
                        ps_ai = psp.tile([128, NPL], F32, tag="ai")
                        for c in range(CT):
                            nc.tensor.matmul(
                                ps_ai[:], featW[:, c, 128 * q : 128 * (q + 1)],
                                ftl[:, c, :], start=(c == 0), stop=(c == CT - 1),
                            )
                        nc.scalar.activation(
                            attn_img[:, q, :], ps_ai[:], ACT.Identity,
                            bias=battn[:, q : q + 1],
                        )

                    # emb_g[g, (t,n)] = W_ih[:, ENC:].T @ embT (+ b_ih + b_hh)
                    for gt in range(16 if DO_PRE else 0):
                        ps_e = psp.tile([128, T * NB], F32, tag="e")
                        for kt in range(4):
                            nc.tensor.matmul(
                                ps_e[:],
                                wie[:, kt, 128 * gt : 128 * (gt + 1)],
                                embl[:, kt, :],
                                start=(kt == 0), stop=(kt == 3),
                            )
                        nc.scalar.activation(
                            emb_g[:, gt, :], ps_e[:], ACT.Identity,
                            bias=behT[:, gt : gt + 1],
                        )

                    # G[(j,p), m, g] = feat @ W_ih[:, :ENC].T (np pair-tiled)
                    for gc in range(4 if DO_PRE else 0):
                        for m in range(NM):
                            ps_g = psp.tile([128, 512], F32, tag="g")
                            for c in range(CT):
                                nc.tensor.matmul(
                                    ps_g[:],
                                    ftl[:, c, 128 * m : 128 * (m + 1)],
                                    wihc[:, c, 512 * gc : 512 * (gc + 1)],
                                    start=(c == 0), stop=(c == CT - 1),
                                )
                            if m % 2 == 0:
                                nc.vector.tensor_copy(
                                    G_sb[:, m, 512 * gc : 512 * (gc + 1)], ps_g[:]
                                )
                            else:
                                nc.scalar.activation(
                                    G_sb[:, m, 512 * gc : 512 * (gc + 1)], ps_g[:],
                                    ACT.Identity,
                                )

            # h0 -> h_all slot 0 (bf16), c0 -> c_a (f32)
            if DO_INIT:
                nc.vector.tensor_copy(h_all[:, :, 0, :], hc_loc[:, 0:4, :])
                for k8 in range(8):
                    nc.vector.tensor_copy(
                        c_a[64 * (k8 % 2) : 64 * (k8 % 2) + 64, k8 // 2, :],
                        c_st[:, k8, :],
                    )
            else:
                nc.vector.memset(h_all[:, :, 0, :], 0.0)
                nc.vector.memset(c_a[:], 0.0)

            # ---------------- loop-persistent + fc weights ----------------
            with tc.tile_pool(name="lp", bufs=1) as lp:
                fcb = lp.tile([1, V], BF16)
                nc.sync.dma_start(fcb[:], fc_bd[:])
                b_rep = lp.tile([128, V], BF16)
                nc.gpsimd.partition_broadcast(b_rep[:], fcb[:])
                fcW = lp.tile([128, 4, V], BF16)
                nc.sync.dma_start(fcW[:], fc_WTd[:])

                # ---------------- recurrence (no collectives) ----------------
                with (
                    tc.tile_pool(name="loop", bufs=2) as pl,
                    tc.tile_pool(name="psl", bufs=2, space="PSUM") as psl,
                ):
                    c_cur, c_nxt = c_a, c_b
                    for t in range(NSTEP if DO_LOOP else 0):
                        h_rhs = h_all[:, :, t, :]  # [128, 4, NB] bf16

                        # misc psum: ah [0:32], sc [32:36], z [40:48], rzb [48:56]
                        ps_m = psl.tile([128, 56], F32, tag="m")
                        ps_ah = ps_m[:, 0:32].rearrange("p (q n) -> p q n", q=4)
                        for q in range(4):
                            for kt in range(4):
                                nc.tensor.matmul(
                                    ps_ah[:, q, :],
                                    tokW[:, kt, 128 * q : 128 * (q + 1)],
                                    h_rhs[:, kt, :],
                                    start=(kt == 0), stop=(kt == 3),
                                )
                        # eh gates: W_hh.T @ h -> [g, n] (early; h-only dep)
                        ps_eh = psl.tile([128, 16, NB], F32, tag="eh")
                        for gt in range(16):
                            for kt in range(4):
                                nc.tensor.matmul(
                                    ps_eh[:, gt, :],
                                    whh[:, kt, 128 * gt : 128 * (gt + 1)],
                                    h_rhs[:, kt, :],
                                    start=(kt == 0), stop=(kt == 3),
                                )
                        # s1 = eh + emb_g[t] off the critical path
                        s1 = pl.tile([128, 16, NB], F32, tag="s1")
                        nc.vector.tensor_tensor(
                            s1[:], ps_eh[:], emb_g[:, :, NB * t : NB * (t + 1)],
                            op=OP.add,
                        )

                        # e = relu(attn_img + attn_h), scores = w . e
                        ah = pl.tile([128, 4, NB], BF16, tag="ah")
                        nc.scalar.activation(ah[:], ps_ah[:], ACT.Identity)
                        e_sb = pl.tile([128, 4, NPL], BF16, tag="e")
                        ps_sc = ps_m[:, 32:36]
                        for q in range(4):
                            epre = pl.tile([128, NB, P], BF16, tag="ep")
                            i0 = attn_img[:, q, :].rearrange(
                                "a (n p) -> a n p", p=P
                            )
                            i1 = ah[:, q, :].rearrange(
                                "a (n one) -> a n one", one=1
                            )
                            i0b, i1b = bass.broadcast_tensor_aps(i0, i1)
                            nc.vector.tensor_tensor(epre[:], i0b, i1b, op=OP.add)
                            nc.scalar.activation(
                                e_sb[:, q, :],
                                epre[:].rearrange("a n p -> a (n p)"),
                                ACT.Relu,
                            )
                        for m in range(NM):
                            for q in range(4):
                                nc.tensor.matmul(
                                    ps_sc[:, m : m + 1],
                                    e_sb[:, q, 128 * m : 128 * (m + 1)],
                                    wfull[:, q : q + 1],
                                    start=(q == 0), stop=(q == 3),
                                )
                        alpha_m = pl.tile([128, NM], BF16, tag="al")
                        nc.scalar.activation(alpha_m[:], ps_sc[:], ACT.Exp)

                        # Z per n via masked ones-matmul; 1/Z; PE partition-bcast
                        ps_z = ps_m[0:1, 40:48].rearrange("p (j m) -> p j m", j=2)
                        for j in range(2):
                            nc.tensor.matmul(
                                ps_z[:, j, :], msk[:, j : j + 1], alpha_m[:],
                                start=True, stop=True,
                            )
                        rz = pl.tile([1, 2, NM], F32, tag="rz")
                        nc.vector.reciprocal(rz[:], ps_z[:])
                        ps_rzb = ps_m[:, 48:56].rearrange("p (j m) -> p j m", j=2)
                        nc.tensor.matmul(
                            ps_rzb[:], ones1[:], rz[:], start=True, stop=True,
                        )
                        nc.vector.tensor_tensor(
                            alpha_pack[0:64, :, 0], alpha_m[0:64, :],
                            ps_rzb[0:64, 0, :], op=OP.mult,
                        )
                        nc.vector.tensor_tensor(
                            alpha_pack[64:128, :, 1], alpha_m[64:128, :],
                            ps_rzb[64:128, 1, :], op=OP.mult,
                        )

                        # ctx gates: alpha @ G -> [g, n] col-pairs
                        ps_cx = psl.tile([128, 16, NB], F32, tag="cx")
                        for gt in range(16):
                            for m in range(NM):
                                nc.tensor.matmul(
                                    ps_cx[:, gt, 2 * m : 2 * m + 2],
                                    G_sb[:, m, 128 * gt : 128 * (gt + 1)],
                                    alpha_pack[:, m, :],
                                    start=True, stop=True,
                                )

                        # gates = s1 + ctx; pointwise LSTM (i|f|g|o)
                        gsum = pl.tile([128, 16, NB], F32, tag="gs")
                        nc.vector.tensor_tensor(
                            gsum[:], s1[:], ps_cx[:], op=OP.add
                        )
                        act_g = pl.tile([128, 16, NB], F32, tag="ag")
                        nc.scalar.activation(
                            act_g[:, 0:8, :], gsum[:, 0:8, :], ACT.Sigmoid
                        )
                        nc.scalar.activation(
                            act_g[:, 8:12, :], gsum[:, 8:12, :], ACT.Tanh
                        )
                        nc.scalar.activation(
                            act_g[:, 12:16, :], gsum[:, 12:16, :], ACT.Sigmoid
                        )
                        a1 = pl.tile([128, 4, NB], F32, tag="a1")
                        nc.vector.tensor_tensor(
                            a1[:], act_g[:, 0:4, :], act_g[:, 8:12, :], op=OP.mult
                        )
                        a2 = pl.tile([128, 4, NB], F32, tag="a2")
                        nc.vector.tensor_tensor(
                            a2[:], act_g[:, 4:8, :], c_cur[:], op=OP.mult
                        )
                        nc.vector.tensor_tensor(c_nxt[:], a1[:], a2[:], op=OP.add)
                        tc_ = pl.tile([128, 4, NB], F32, tag="tc")
                        nc.scalar.activation(tc_[:], c_nxt[:], ACT.Tanh)
                        nc.vector.tensor_tensor(
                            h_all[:, :, t + 1, :], act_g[:, 12:16, :], tc_[:],
                            op=OP.mult,
                        )
                        c_cur, c_nxt = c_nxt, c_cur

                # ---------------- fc: preds[(t,n), v] ----------------
                vcs = [(i * 512, min(V, (i + 1) * 512)) for i in range((V + 511) // 512)]
                with (
                    tc.tile_pool(name="fcl", bufs=3) as pfc,
                    tc.tile_pool(name="psf", bufs=2, space="PSUM") as psf,
                ):
                    for j in range(2 if DO_FC else 0):
                        for lo, hi in vcs:
                            w = hi - lo
                            ps_f = psf.tile([128, 512], F32, tag="f")
                            for kt in range(4):
                                nc.tensor.matmul(
                                    ps_f[:, 0:w],
                                    h_all[:, kt, 1 + 16 * j : 17 + 16 * j, :],
                                    fcW[:, kt, lo:hi],
                                    start=(kt == 0), stop=(kt == 3),
                                )
                            ob = pfc.tile([128, 512], F32, tag="ob")
                            nc.vector.scalar_tensor_tensor(
                                ob[:, 0:w], ps_f[:, 0:w], 1.0, b_rep[:, lo:hi],
                                op0=OP.mult, op1=OP.add,
                            )
                            nc.sync.dma_start(
                                preds_out[128 * j : 128 * (j + 1), lo:hi],
                                ob[:, 0:w],
                            )

    nc.compile()
    return nc


def _prep(inputs):
    """Host-side layout prep (slicing/transposes/dtype casts only)."""
    f = np.ascontiguousarray(np.asarray(inputs["features"], np.float32))
    cap = np.asarray(inputs["captions"])
    embd_W = np.asarray(inputs["embd_W"], np.float32)
    tokW = np.asarray(inputs["attn_token_W"], np.float32)
    tokb = np.asarray(inputs["attn_token_b"], np.float32)
    featW = np.asarray(inputs["attn_feat_W"], np.float32)
    featb = np.asarray(inputs["attn_feat_b"], np.float32)
    wfull = np.asarray(inputs["attn_full_W"], np.float32)
    W_ih = np.asarray(inputs["W_ih"], np.float32)
    b_ih = np.asarray(inputs["b_ih"], np.float32)
    W_hh = np.asarray(inputs["W_hh"], np.float32)
    b_hh = np.asarray(inputs["b_hh"], np.float32)
    fc_W = np.asarray(inputs["fc_W"], np.float32)
    fc_b = np.asarray(inputs["fc_b"], np.float32)
    iWh = np.asarray(inputs["init_Wh"], np.float32)
    iWc = np.asarray(inputs["init_Wc"], np.float32)

    def t128(x, pdim):  # (pdim*128, rest...) -> [128, pdim, rest]
        return np.ascontiguousarray(
            x.reshape(pdim, 128, *x.shape[1:]).transpose(1, 0, *range(2, x.ndim + 1))
        )

    # replicated weights (same on every core)
    w_ihcT = t128(W_ih[:, :ENC].T.copy(), CT).astype(bf16)   # [128,12,2048]
    w_ieT = t128(W_ih[:, ENC:].T.copy(), 4).astype(bf16)     # [128,4,2048]
    w_hhT = t128(W_hh.T.copy(), 4).astype(bf16)              # [128,4,2048]
    b_ehT = np.ascontiguousarray(
        (b_ih + b_hh).reshape(16, 128).T
    ).astype(np.float32)                                      # [128,16]
    tok_WT = t128(tokW.T.copy(), 4).astype(bf16)             # [128,4,512]
    feat_WT = t128(featW.T.copy(), CT).astype(bf16)          # [128,12,512]
    b_attn = np.ascontiguousarray((tokb + featb).reshape(4, 128).T).astype(np.float32)
    w_full8 = np.ascontiguousarray(wfull[0].reshape(4, 128).T).astype(bf16)
    masks = np.zeros((128, 2), np.float32)
    masks[:64, 0] = 1.0
    masks[64:, 1] = 1.0
    masks = masks.astype(bf16)
    fc_WT = t128(fc_W.T.copy(), 4).astype(bf16)              # [128,4,10000]
    fc_b8 = fc_b.reshape(1, V).astype(bf16)
    ones1 = np.ones((1, 128), np.float32)
    ident = np.eye(128, dtype=np.float32)

    in_maps = []
    for k in range(NC_):
        rows = slice(NB * k, NB * (k + 1))
        floc = np.ascontiguousarray(f[rows])                  # (8, 64, 1536)
        featT_loc = t128(floc.reshape(NPL, ENC).T.copy(), CT).astype(bf16)
        # emb tokens for local rows: [E, (t, n)]
        emb_tok = embd_W[cap[rows]]                           # (8, 32, 512)
        embT_loc = t128(
            np.ascontiguousarray(emb_tok.transpose(2, 1, 0)).reshape(E, T * NB), 4
        ).astype(bf16)
        # init shard: pixel slice [NB*k, NB*(k+1)) across all batch rows
        fpix = np.ascontiguousarray(f[:, rows, :])            # (64, 8, 1536)
        feat_initT = t128(fpix.reshape(N, KS).T.copy(), 96).astype(bf16)
        initW = t128(
            np.concatenate(
                [iWh[KS * k : KS * (k + 1)], iWc[KS * k : KS * (k + 1)]], axis=1
            ), 96
        ).astype(bf16)
        in_maps.append({
            "featT_loc": featT_loc, "w_ihcT": w_ihcT, "w_ieT": w_ieT,
            "w_hhT": w_hhT, "b_ehT": b_ehT, "embT_loc": embT_loc,
            "tok_WT": tok_WT, "feat_WT": feat_WT, "b_attn": b_attn,
            "w_full": w_full8, "masks": masks,
            "feat_initT": feat_initT, "initW": initW,
            "fc_WT": fc_WT, "fc_b": fc_b8, "ones1": ones1, "ident": ident,
        })
    return in_maps


def kernel(**inputs) -> np.ndarray:
    if "nc" not in _cache:
        _cache["nc"] = _build()
    nc = _cache["nc"]
    in_maps = _prep(inputs)
    res = run_bass_kernel_spmd(nc, in_maps, core_ids=list(range(NC_)), trace=False)
    parts = [
        r["preds"].reshape(T, NB, V).transpose(1, 0, 2) for r in res.results
    ]
    return np.ascontiguousarray(np.concatenate(parts, axis=0))
